# revision 5
# baseline (speedup 1.0000x reference)
"""DeformConv2d (B=8, C=128, H=W=64, K=3x3, pad 1, stride 1) on 8 trn2 NeuronCores.

Data-parallel over batch: core b handles image b. Per core:
  - Host packs x into P4: for each padded pixel i, one fp32 word pair holding
    bf16 lanes (XP[i], XP[i+68], XP[i+1], XP[i+69]) -- the 2x2 bilinear patch.
    One gpsimd ap_gather index fetches all 4 corners (d=2 fp32 words).
  - Host pre-transposes offsets to [128, 2*K*Q] (position%128 on partitions)
    so index/weight math runs directly on DVE with zero transpose DMAs.
  - Bilinear corner weights (4 bf16 lanes per tap-position) are computed on
    DVE compactly, staged per-tap to DRAM rows, and broadcast to all 128
    partitions with stride-0-source DMAs round-robined over the two HWDGE
    rings (sync/scalar), 512KB per (eighth, tap).
  - Per (eighth, tap): gather (gpsimd), weight multiply (DVE, bf16 2x),
    4x 512-col matmuls (PE) accumulating 9 taps into a 4-bank PSUM tile.
  - Tail per eighth: sum 4 corner lanes + bias -> fp32 out, stored via the
    gpsimd SWDGE ring to keep HWDGE rings free for weight broadcasts.
"""
import numpy as np
import ml_dtypes

B, CIN, H, W = 8, 128, 64, 64
COUT, KH, KW = 128, 3, 3
K = KH * KW
HO, WO = 64, 64
P = 128                      # partitions
NPOS = HO * WO               # 4096 output positions per image
Q = NPOS // P                # 32 position-blocks of 128
PADR = 2                     # zero-pad ring width
HP = H + 2 * PADR            # 68
WP = W + 2 * PADR            # 68
NE = HP * WP                 # 4624 padded elements
NXP = NE + WP + 1            # padded alloc with tail zeros for corner shifts
NE8 = 8                      # eighths
NQ8 = NPOS // NE8            # 512 positions per eighth
QI8 = NQ8 // 16              # 32 idx-cols per eighth per tap
FB = 1024.0                  # floor-trick bias constant


def _build_kernel():
    import concourse.bacc as bacc
    import concourse.mybir as mybir
    import concourse.tile as tile
    import concourse.library_config as library_config

    nc = bacc.Bacc("TRN2", target_bir_lowering=False, debug=False, num_devices=8)
    f32, bf16, i16 = mybir.dt.float32, mybir.dt.bfloat16, mybir.dt.int16
    ALU = mybir.AluOpType

    p4_d = nc.dram_tensor("p4", [P, 2 * NE], f32, kind="ExternalInput")
    offt_d = nc.dram_tensor("offt", [P, 2 * K * Q], f32, kind="ExternalInput")
    hw_d = nc.dram_tensor("hobwob", [P, 2 * K * Q], f32, kind="ExternalInput")
    wmat_d = nc.dram_tensor("wmat", [P, K * COUT], bf16, kind="ExternalInput")
    bias_d = nc.dram_tensor("bias", [P, 1], f32, kind="ExternalInput")
    out_d = nc.dram_tensor("out", [P, NPOS], f32, kind="ExternalOutput")

    NG = K * Q  # 288

    with tile.TileContext(nc) as tc:
        with tc.tile_pool(name="const", bufs=1) as cpool, \
             tc.tile_pool(name="gen", bufs=1) as gpool, \
             tc.tile_pool(name="wbc", bufs=5) as wpool, \
             tc.tile_pool(name="gath", bufs=4) as gapool, \
             tc.tile_pool(name="mm", bufs=3) as mpool, \
             tc.tile_pool(name="outp", bufs=2) as opool, \
             tc.tile_pool(name="dramw", bufs=1, space="DRAM") as dpool, \
             tc.tile_pool(name="ps", bufs=2, space="PSUM") as pspool:

            wrow = dpool.tile([K, 4 * NPOS], mybir.dt.bfloat16)

            nc.gpsimd.load_library(library_config.ap_gather)

            # ---------------- input loads ------------------------------
            offt = cpool.tile([P, 2 * NG], f32)
            nc.sync.dma_start(out=offt[:], in_=offt_d.ap())
            hw = cpool.tile([P, 2 * NG], f32)
            nc.scalar.dma_start(out=hw[:], in_=hw_d.ap())
            P4 = cpool.tile([P, 2 * NE], f32)
            nc.sync.dma_start(out=P4[:, 0:NE], in_=p4_d.ap()[:, 0:NE])
            nc.scalar.dma_start(out=P4[:, NE:2 * NE], in_=p4_d.ap()[:, NE:2 * NE])
            wmat = cpool.tile([P, K * COUT], bf16)
            nc.scalar.dma_start(out=wmat[:], in_=wmat_d.ap())
            bias = cpool.tile([P, 1], f32)
            nc.sync.dma_start(out=bias[:], in_=bias_d.ap())

            # ---------------- index + weight math (compact) ------------
            pyb = gpool.tile([P, NG], f32)
            pxb = gpool.tile([P, NG], f32)
            nc.vector.scalar_tensor_tensor(
                out=pyb[:], in0=offt[:, 0:NG], scalar=FB, in1=hw[:, 0:NG],
                op0=ALU.add, op1=ALU.add)
            nc.vector.scalar_tensor_tensor(
                out=pxb[:], in0=offt[:, NG:2 * NG], scalar=FB, in1=hw[:, NG:2 * NG],
                op0=ALU.add, op1=ALU.add)

            # floor robust to cast rounding mode (trunc in sim, RN on hw)
            def floor_frac(pb, sfx):
                i0 = gpool.tile([P, NG], mybir.dt.int32, tag="ffi" + sfx)
                nc.vector.tensor_copy(out=i0[:], in_=pb[:])
                f0 = gpool.tile([P, NG], f32, tag="fff" + sfx)
                nc.vector.tensor_copy(out=f0[:], in_=i0[:])
                lr = gpool.tile([P, NG], f32, tag="ffl" + sfx)
                nc.vector.tensor_tensor(out=lr[:], in0=pb[:], in1=f0[:],
                                        op=ALU.subtract)
                adj = gpool.tile([P, NG], f32, tag="ffa" + sfx)
                nc.vector.tensor_scalar(out=adj[:], in0=lr[:], scalar1=0.0,
                                        scalar2=None, op0=ALU.is_lt)
                fr = gpool.tile([P, NG], f32, tag="ffr" + sfx)
                nc.vector.tensor_tensor(out=fr[:], in0=lr[:], in1=adj[:],
                                        op=ALU.add)
                fl = gpool.tile([P, NG], f32, tag="ffo" + sfx)
                nc.vector.tensor_tensor(out=fl[:], in0=f0[:], in1=adj[:],
                                        op=ALU.subtract)
                return fl, fr

            y0f, ly = floor_frac(pyb, "y")
            x0f, lx = floor_frac(pxb, "x")
            omly = gpool.tile([P, NG], f32)
            omlx = gpool.tile([P, NG], f32)
            nc.vector.tensor_scalar(out=omly[:], in0=ly[:], scalar1=-1.0,
                                    scalar2=1.0, op0=ALU.mult, op1=ALU.add)
            nc.vector.tensor_scalar(out=omlx[:], in0=lx[:], scalar1=-1.0,
                                    scalar2=1.0, op0=ALU.mult, op1=ALU.add)
            ycl = gpool.tile([P, NG], f32)
            xcl = gpool.tile([P, NG], f32)
            nc.vector.tensor_scalar(out=ycl[:], in0=y0f[:], scalar1=FB - PADR,
                                    scalar2=FB + 64.0, op0=ALU.max, op1=ALU.min)
            nc.vector.tensor_scalar(out=xcl[:], in0=x0f[:], scalar1=FB - PADR,
                                    scalar2=FB + 64.0, op0=ALU.max, op1=ALU.min)
            linf = gpool.tile([P, NG], f32)
            nc.vector.scalar_tensor_tensor(
                out=linf[:], in0=ycl[:], scalar=float(WP), in1=xcl[:],
                op0=ALU.mult, op1=ALU.add)
            linf2 = gpool.tile([P, NG], f32)
            nc.vector.tensor_scalar(out=linf2[:], in0=linf[:],
                                    scalar1=-(WP + 1.0) * (FB - PADR),
                                    scalar2=None, op0=ALU.add)
            lin16 = gpool.tile([P, NG], i16)
            nc.vector.tensor_copy(out=lin16[:], in_=linf2[:])

            # bilinear weight products, 4-lane interleave matching P4 lanes
            wpre_cat = gpool.tile([P, 4 * NG], bf16)
            wv = wpre_cat[:].rearrange("p (k q j) -> p k q j", k=K, q=Q, j=4)
            omly3 = omly[:].rearrange("p (k q) -> p k q", k=K, q=Q)
            ly3 = ly[:].rearrange("p (k q) -> p k q", k=K, q=Q)
            omlx3 = omlx[:].rearrange("p (k q) -> p k q", k=K, q=Q)
            lx3 = lx[:].rearrange("p (k q) -> p k q", k=K, q=Q)
            nc.vector.tensor_tensor(out=wv[:, :, :, 0], in0=omly3, in1=omlx3,
                                    op=ALU.mult)  # w00 (A)
            nc.vector.tensor_tensor(out=wv[:, :, :, 1], in0=ly3, in1=omlx3,
                                    op=ALU.mult)  # w10 (C)
            nc.vector.tensor_tensor(out=wv[:, :, :, 2], in0=omly3, in1=lx3,
                                    op=ALU.mult)  # w01 (B)
            nc.vector.tensor_tensor(out=wv[:, :, :, 3], in0=ly3, in1=lx3,
                                    op=ALU.mult)  # w11 (D)

            # stage per-tap rows: wrow[k, (q*128+Pp)*4 + j] = wpre_cat[Pp, kqj]
            wrow_v = wrow[:].rearrange("k (q p j) -> p k q j", k=K, q=Q, p=P, j=4)
            wpre_v = wpre_cat[:].rearrange("p (k q j) -> p k q j", k=K, q=Q, j=4)
            for k in range(K):
                eng = nc.sync if k % 2 == 0 else nc.scalar
                eng.dma_start(out=wrow_v[:, k], in_=wpre_v[:, k])

            # gather index tensor: wrapped-16 layout for ap_gather
            NI = 8 * K * Q  # 2304 idx-cols total (256 per tap)
            idxw = gpool.tile([P, NI], i16)
            for u in range(8):
                eng = nc.sync if u % 2 == 0 else nc.scalar
                eng.dma_start(
                    out=idxw[0:16, :].rearrange(
                        "p (k q u) -> p k q u", k=K, q=Q, u=8)[:, :, :, u],
                    in_=lin16[16 * u: 16 * u + 16, :].rearrange(
                        "p (k q) -> p k q", k=K, q=Q),
                )
            nc.sync.dma_start(out=idxw[16:32, :], in_=idxw[0:16, :])
            nc.scalar.dma_start(out=idxw[32:64, :], in_=idxw[0:32, :])
            nc.sync.dma_start(out=idxw[64:128, :], in_=idxw[0:64, :])

            # ---------------- main loop: quarters x taps ----------------
            # bcast + gather + mult at (qt, k) granularity; matmuls split
            # into two half-psum tiles (4 banks each) so tails double-buffer.
            NQT = NPOS // 4   # 1024 positions per quarter
            QI = NQT // 16    # 64 idx-cols per quarter per tap
            rr = 0
            for qt in range(4):
                psA = pspool.tile([P, 2 * NQT], mybir.dt.float32, tag="ps")
                psB = pspool.tile([P, 2 * NQT], mybir.dt.float32, tag="ps")
                ps = [psA, psB]
                for k in range(K):
                    sl = slice(qt * 4 * NQT, (qt + 1) * 4 * NQT)
                    i0 = k * (8 * Q) + qt * QI
                    with tc.high_priority():
                        wb = wpool.tile([P, 4 * NQT], bf16, tag="wb")
                        eng = nc.sync if rr % 2 == 0 else nc.scalar
                        rr += 1
                        eng.dma_start(
                            out=wb[:],
                            in_=wrow[k: k + 1, sl].to_broadcast((P, 4 * NQT)))
                        g4 = gapool.tile([P, 2 * NQT], f32, tag="g")
                        nc.gpsimd.ap_gather(
                            g4[:], P4[:], idxw[:, i0: i0 + QI],
                            channels=P, num_elems=NE, d=2, num_idxs=NQT)
                    m = mpool.tile([P, 4 * NQT], bf16, tag="m")
                    nc.vector.tensor_tensor(
                        out=m[:], in0=g4[:].bitcast(bf16), in1=wb[:],
                        op=ALU.mult)
                    lhsT = wmat[:, k * COUT: (k + 1) * COUT]
                    for j in range(8):
                        h, bk = j // 4, j % 4
                        c0 = bk * 512
                        nc.tensor.matmul(
                            ps[h][:, c0: c0 + 512], lhsT,
                            m[:, h * 2 * NQT + c0: h * 2 * NQT + c0 + 512],
                            start=(k == 0), stop=(k == K - 1),
                            skip_group_check=True)
                # tails: sum 4 corner lanes + bias -> fp32 out (per half)
                for h in range(2):
                    e = 2 * qt + h
                    pv = ps[h][:].rearrange("p (n j) -> p n j", j=4)
                    with tc.high_priority():
                        t = opool.tile([P, NQ8], f32, tag="t")
                        nc.vector.tensor_scalar(
                            out=t[:], in0=pv[:, :, 0],
                            scalar1=bias[:, 0:1], scalar2=None, op0=ALU.add)
                        t2 = opool.tile([P, NQ8], f32, tag="t2")
                        nc.vector.tensor_tensor(
                            out=t2[:], in0=t[:], in1=pv[:, :, 1], op=ALU.add)
                        t3 = opool.tile([P, NQ8], f32, tag="t3")
                        nc.vector.tensor_tensor(
                            out=t3[:], in0=t2[:], in1=pv[:, :, 2], op=ALU.add)
                        o = opool.tile([P, NQ8], f32, tag="o")
                        nc.vector.tensor_tensor(
                            out=o[:], in0=t3[:], in1=pv[:, :, 3], op=ALU.add)
                    nc.gpsimd.dma_start(
                        out=out_d.ap()[:, e * NQ8: (e + 1) * NQ8], in_=o[:])

    nc.compile()
    return nc


_NC_CACHE = None


def _host_inputs(x, offset, weight, bias):
    """Per-core input maps (core b <- batch b) + replicated constants."""
    wq = np.ascontiguousarray(weight, np.float32)  # [COUT, CIN, KH, KW]
    # wmat[c, k*COUT + o] = weight[o, c, ky, kx]
    wmat = wq.reshape(COUT, CIN, K).transpose(1, 2, 0).reshape(CIN, K * COUT)
    wmat = np.ascontiguousarray(wmat).astype(ml_dtypes.bfloat16)
    bias_h = np.ascontiguousarray(bias, np.float32).reshape(P, 1)
    # hob[Pp, k*Q+q] = ho(p) - 1 + ky,  wob = wo(p) - 1 + kx,  p = q*128 + Pp
    p_of = (np.arange(Q)[:, None] * P + np.arange(P)[None, :])  # [Q, P]
    ho = (p_of // WO).astype(np.float32)
    wo = (p_of % WO).astype(np.float32)
    hobwob = np.empty((P, 2 * K * Q), np.float32)
    for k in range(K):
        hobwob[:, k * Q: (k + 1) * Q] = (ho + (k // 3 - 1)).T
        hobwob[:, K * Q + k * Q: K * Q + (k + 1) * Q] = (wo + (k % 3 - 1)).T
    in_maps = []
    for b in range(B):
        xb = np.ascontiguousarray(x[b], np.float32).reshape(P, H, W)
        XP = np.zeros((P, NXP), np.float32)
        XP[:, 0:NE].reshape(P, HP, WP)[:, PADR:PADR + H, PADR:PADR + W] = xb
        p4h = np.empty((P, 4 * NE), ml_dtypes.bfloat16)
        p4h[:, 0::4] = XP[:, 0:NE]
        p4h[:, 1::4] = XP[:, WP:NE + WP]
        p4h[:, 2::4] = XP[:, 1:NE + 1]
        p4h[:, 3::4] = XP[:, WP + 1:NE + WP + 1]
        p4 = p4h.view(np.float32)  # [P, 2*NE]
        offb = np.ascontiguousarray(offset[b], np.float32).reshape(2 * K, NPOS)
        offt = np.empty((P, 2 * K * Q), np.float32)
        for k in range(K):
            offt[:, k * Q: (k + 1) * Q] = offb[2 * k].reshape(Q, P).T
            offt[:, K * Q + k * Q: K * Q + (k + 1) * Q] = \
                offb[2 * k + 1].reshape(Q, P).T
        in_maps.append({
            "p4": p4,
            "offt": offt,
            "hobwob": hobwob,
            "wmat": wmat,
            "bias": bias_h,
        })
    return in_maps


def kernel(x, offset, weight, bias):
    global _NC_CACHE
    from concourse.bass_utils import run_bass_kernel_spmd

    if _NC_CACHE is None:
        _NC_CACHE = _build_kernel()
    nc = _NC_CACHE
    in_maps = _host_inputs(x, offset, weight, bias)
    res = run_bass_kernel_spmd(nc, in_maps, list(range(B)))
    out = np.stack([res.results[b]["out"].reshape(COUT, HO, WO) for b in range(B)])
    return out.astype(np.float32)


if __name__ == "__main__":
    import sys
    d = np.load("/tmp/inputs.npz")
    if len(sys.argv) > 1 and sys.argv[1] == "sim":
        from concourse.bass_interp import CoreSim
        nc = _build_kernel()
        in_maps = _host_inputs(d["x"], d["offset"], d["weight"], d["bias"])
        sim = CoreSim(nc)
        for kk, vv in in_maps[0].items():
            sim.tensor(kk)[:] = vv
        sim.simulate()
        out = np.asarray(sim.tensor("out")).reshape(1, COUT, HO, WO)
        exp = np.load("/tmp/expected.npy")[0:1]
    else:
        out = kernel(d["x"], d["offset"], d["weight"], d["bias"])
        exp = np.load("/tmp/expected.npy")
    err = np.abs(out - exp)
    print("rel l2:", np.linalg.norm(out - exp) / np.linalg.norm(exp))
    print("absmax rel:", err.max() / np.abs(exp).max())


# revision 12
# speedup vs baseline: 3.2006x; 3.2006x over previous
"""DeformConv2d (B=8, C=128, H=W=64, K=3x3, pad 1, stride 1) on 8 trn2 NeuronCores.

Data-parallel over batch: core b handles image b. Per core:
  - Host packs x into a position-major DRAM image xpm[NE, 512] bf16: row i
    holds the 2x2 bilinear patch at padded pixel i as 4 channel-blocks
    [A=x(i), C=x(i+68), B=x(i+1), D=x(i+69)] x 128 channels.
  - Host pre-transposes offsets to [128, K*Q] pairs so index/weight math runs
    on DVE with zero transpose DMAs.
  - Indices: lin16 computed on DVE, staged via DRAM, and shuffled into the
    wrapped-16 idx layout for dma_gather (stream i = 8*p + q).
  - Per (quarter, tap): SWDGE dma_gather (transpose mode, 4 queues) fetches
    1024 rows of 1KB into [128ch, 4 corner-blocks, 1024 pos] bf16; bilinear
    corner weights (computed compactly, staged per-tap to DRAM in matching
    (j, p, q) order) are broadcast stride-0 on the HWDGE rings; DVE multiply;
    PE matmuls accumulate 9 taps into two position-half PSUM tiles.
  - Tail per (quarter, half): sum 4 corner lanes + bias -> fp32 out.
"""
import numpy as np
import ml_dtypes

B, CIN, H, W = 8, 128, 64, 64
COUT, KH, KW = 128, 3, 3
K = KH * KW
HO, WO = 64, 64
P = 128                      # partitions
NPOS = HO * WO               # 4096 output positions per image
Q = NPOS // P                # 32 position-blocks of 128
PADR = 2                     # zero-pad ring width
HP = H + 2 * PADR            # 68
WP = W + 2 * PADR            # 68
NE = HP * WP                 # 4624 padded pixels
ES = 4 * P                   # 512 bf16 elements per xpm row (1KB)
NQT = NPOS // 4              # 1024 positions per quarter
NH = NQT // 2                # 512 positions per half
FB = 1024.0                  # floor-trick bias constant


def _build_kernel():
    import concourse.bacc as bacc
    import concourse.mybir as mybir
    import concourse.tile as tile
    import concourse.library_config as library_config

    nc = bacc.Bacc("TRN2", target_bir_lowering=False, debug=False,
                   num_devices=8, num_swdge_queues=4)
    f32, bf16, i16 = mybir.dt.float32, mybir.dt.bfloat16, mybir.dt.int16
    ALU = mybir.AluOpType

    xpm_d = nc.dram_tensor("xpm", [NE, ES], bf16, kind="ExternalInput")
    offt_d = nc.dram_tensor("offt", [P, 2 * K * Q], f32, kind="ExternalInput")
    hw_d = nc.dram_tensor("hobwob", [P, 2 * K * Q], f32, kind="ExternalInput")
    wmat_d = nc.dram_tensor("wmat", [P, K * COUT], bf16, kind="ExternalInput")
    bias_d = nc.dram_tensor("bias", [P, 1], f32, kind="ExternalInput")
    out_d = nc.dram_tensor("out", [P, NPOS], f32, kind="ExternalOutput")

    NG = K * Q  # 288
    NI0 = 4 * K * 64  # 2304 cols of the wrapped idx block

    with tile.TileContext(nc) as tc:
        with tc.tile_pool(name="const", bufs=1) as cpool, \
             tc.tile_pool(name="gen", bufs=1) as gpool, \
             tc.tile_pool(name="wbc", bufs=5) as wpool, \
             tc.tile_pool(name="gath", bufs=4) as gapool, \
             tc.tile_pool(name="mm", bufs=3) as mpool, \
             tc.tile_pool(name="outp", bufs=2) as opool, \
             tc.tile_pool(name="dramw", bufs=1, space="DRAM") as dpool, \
             tc.tile_pool(name="ps", bufs=2, space="PSUM") as pspool:

            wrow = dpool.tile([K, 4 * NPOS], mybir.dt.bfloat16)
            linD = dpool.tile([16, NI0], i16)

            nc.gpsimd.load_library(library_config.mlp)

            # ---------------- input loads ------------------------------
            offt = cpool.tile([P, 2 * NG], f32)
            nc.sync.dma_start(out=offt[:], in_=offt_d.ap())
            hw = cpool.tile([P, 2 * NG], f32)
            nc.scalar.dma_start(out=hw[:], in_=hw_d.ap())
            wmat = cpool.tile([P, K * COUT], bf16)
            nc.scalar.dma_start(out=wmat[:], in_=wmat_d.ap())
            bias = cpool.tile([P, 1], f32)
            nc.sync.dma_start(out=bias[:], in_=bias_d.ap())

            # ---------------- index + weight math (compact) ------------
            pyb = gpool.tile([P, NG], f32)
            pxb = gpool.tile([P, NG], f32)
            nc.vector.scalar_tensor_tensor(
                out=pyb[:], in0=offt[:, 0:NG], scalar=FB, in1=hw[:, 0:NG],
                op0=ALU.add, op1=ALU.add)
            nc.vector.scalar_tensor_tensor(
                out=pxb[:], in0=offt[:, NG:2 * NG], scalar=FB, in1=hw[:, NG:2 * NG],
                op0=ALU.add, op1=ALU.add)

            def floor_frac(pb, sfx):
                i0 = gpool.tile([P, NG], mybir.dt.int32, tag="ffi" + sfx)
                nc.vector.tensor_copy(out=i0[:], in_=pb[:])
                f0 = gpool.tile([P, NG], f32, tag="fff" + sfx)
                nc.vector.tensor_copy(out=f0[:], in_=i0[:])
                lr = gpool.tile([P, NG], f32, tag="ffl" + sfx)
                nc.vector.tensor_tensor(out=lr[:], in0=pb[:], in1=f0[:],
                                        op=ALU.subtract)
                adj = gpool.tile([P, NG], f32, tag="ffa" + sfx)
                nc.vector.tensor_scalar(out=adj[:], in0=lr[:], scalar1=0.0,
                                        scalar2=None, op0=ALU.is_lt)
                fr = gpool.tile([P, NG], f32, tag="ffr" + sfx)
                nc.vector.tensor_tensor(out=fr[:], in0=lr[:], in1=adj[:],
                                        op=ALU.add)
                fl = gpool.tile([P, NG], f32, tag="ffo" + sfx)
                nc.vector.tensor_tensor(out=fl[:], in0=f0[:], in1=adj[:],
                                        op=ALU.subtract)
                return fl, fr

            y0f, ly = floor_frac(pyb, "y")
            x0f, lx = floor_frac(pxb, "x")
            omly = gpool.tile([P, NG], f32)
            omlx = gpool.tile([P, NG], f32)
            nc.vector.tensor_scalar(out=omly[:], in0=ly[:], scalar1=-1.0,
                                    scalar2=1.0, op0=ALU.mult, op1=ALU.add)
            nc.vector.tensor_scalar(out=omlx[:], in0=lx[:], scalar1=-1.0,
                                    scalar2=1.0, op0=ALU.mult, op1=ALU.add)
            ycl = gpool.tile([P, NG], f32)
            xcl = gpool.tile([P, NG], f32)
            nc.vector.tensor_scalar(out=ycl[:], in0=y0f[:], scalar1=FB - PADR,
                                    scalar2=FB + 64.0, op0=ALU.max, op1=ALU.min)
            nc.vector.tensor_scalar(out=xcl[:], in0=x0f[:], scalar1=FB - PADR,
                                    scalar2=FB + 64.0, op0=ALU.max, op1=ALU.min)
            linf = gpool.tile([P, NG], f32)
            nc.vector.scalar_tensor_tensor(
                out=linf[:], in0=ycl[:], scalar=float(WP), in1=xcl[:],
                op0=ALU.mult, op1=ALU.add)
            linf2 = gpool.tile([P, NG], f32)
            nc.vector.tensor_scalar(out=linf2[:], in0=linf[:],
                                    scalar1=-(WP + 1.0) * (FB - PADR),
                                    scalar2=None, op0=ALU.add)
            lin16 = gpool.tile([P, NG], i16)
            nc.vector.tensor_copy(out=lin16[:], in_=linf2[:])

            # idx shuffle via DRAM: idxw[8*pm+q, k*256+qt*64+ph]
            #   = lin16[2*ph+pm, k*32+qt*8+q]
            # lin16 -> linF (flat DRAM) -> L[pm, q, kq, ph] -> idxw reads.
            NI = 8 * NG  # 2304 idx-cols (256 per tap: 64 per (qt,k))
            idxw = gpool.tile([P, NI], i16)
            linF = dpool.tile([P, NG], i16)
            nc.sync.dma_start(out=linF[:], in_=lin16[:])
            L_v = linD[:].rearrange("(pm q) (kq ph) -> pm q kq ph",
                                    pm=2, q=8, kq=4 * K, ph=64)
            linF_v = linF[:].rearrange("(ph pm) (kq q) -> ph pm kq q",
                                       ph=64, pm=2, kq=4 * K, q=8)
            rr0 = 0
            for pm in range(2):
                for q in range(8):
                    eng = nc.sync if rr0 % 2 == 0 else nc.scalar
                    rr0 += 1
                    eng.dma_start(
                        out=L_v[pm, q].rearrange("kq ph -> ph kq"),
                        in_=linF_v[:, pm, :, q])
            for pm in range(2):
                eng = nc.sync if pm == 0 else nc.scalar
                eng.dma_start(
                    out=idxw[8 * pm: 8 * pm + 8, :].rearrange(
                        "p (kq ph) -> p kq ph", kq=4 * K, ph=64),
                    in_=L_v[pm])
            nc.sync.dma_start(out=idxw[16:32, :], in_=idxw[0:16, :])
            nc.scalar.dma_start(out=idxw[32:64, :], in_=idxw[0:32, :])
            nc.sync.dma_start(out=idxw[64:128, :], in_=idxw[0:64, :])

            # bilinear weight products: wpre2[p, (k, qt, j, q)] bf16
            wpre2 = gpool.tile([P, 4 * NG], bf16)
            wv = wpre2[:].rearrange("p (k qt j q) -> p k qt j q",
                                    k=K, qt=4, j=4, q=8)
            omly3 = omly[:].rearrange("p (k qt q) -> p k qt q", k=K, qt=4, q=8)
            ly3 = ly[:].rearrange("p (k qt q) -> p k qt q", k=K, qt=4, q=8)
            omlx3 = omlx[:].rearrange("p (k qt q) -> p k qt q", k=K, qt=4, q=8)
            lx3 = lx[:].rearrange("p (k qt q) -> p k qt q", k=K, qt=4, q=8)
            nc.vector.tensor_tensor(out=wv[:, :, :, 0], in0=omly3, in1=omlx3,
                                    op=ALU.mult)  # w00 (A)
            nc.vector.tensor_tensor(out=wv[:, :, :, 1], in0=ly3, in1=omlx3,
                                    op=ALU.mult)  # w10 (C)
            nc.vector.tensor_tensor(out=wv[:, :, :, 2], in0=omly3, in1=lx3,
                                    op=ALU.mult)  # w01 (B)
            nc.vector.tensor_tensor(out=wv[:, :, :, 3], in0=ly3, in1=lx3,
                                    op=ALU.mult)  # w11 (D)

            # stage per-tap rows: wrow[k, qt*4096 + j*1024 + p*8 + q]
            wpre_v = wpre2[:].rearrange("p (k qt j q) -> p k qt j q",
                                        k=K, qt=4, j=4, q=8)
            for k in range(K):
                eng = nc.sync if k % 2 == 0 else nc.scalar
                eng.dma_start(
                    out=wrow[k: k + 1, :].rearrange(
                        "k (qt j p q) -> (k p) qt j q", qt=4, j=4, p=P, q=8),
                    in_=wpre_v[:, k])

            # ---------------- main loop: quarters x taps ----------------
            rr = 0
            for qt in range(4):
                psA = pspool.tile([P, 4 * NH], mybir.dt.float32, tag="ps")
                psB = pspool.tile([P, 4 * NH], mybir.dt.float32, tag="ps")
                ps = [psA, psB]
                for k in range(K):
                    sl = slice(qt * 4 * NQT, (qt + 1) * 4 * NQT)
                    with tc.high_priority():
                        wb = wpool.tile([P, 4 * NQT], bf16, tag="wb")
                        eng = nc.sync if rr % 2 == 0 else nc.scalar
                        eng.dma_start(
                            out=wb[:],
                            in_=wrow[k: k + 1, sl].to_broadcast((P, 4 * NQT)))
                        g4 = gapool.tile([P, 4 * NQT], bf16, tag="g")
                        i0 = k * 256 + qt * 64
                        nc.gpsimd.dma_gather(
                            g4[:].rearrange("p (b n) -> p b n", b=4),
                            xpm_d.ap(), idxw[:, i0: i0 + 64],
                            num_idxs=NQT, num_idxs_reg=NQT,
                            elem_size=ES, transpose=True,
                            queue_num=rr % 4, single_packet=False)
                    rr += 1
                    m = mpool.tile([P, 4 * NQT], bf16, tag="m")
                    nc.vector.tensor_tensor(
                        out=m[:], in0=g4[:], in1=wb[:], op=ALU.mult)
                    lhsT = wmat[:, k * COUT: (k + 1) * COUT]
                    for j in range(4):
                        for h in range(2):
                            nc.tensor.matmul(
                                ps[h][:, j * NH: (j + 1) * NH], lhsT,
                                m[:, j * NQT + h * NH: j * NQT + (h + 1) * NH],
                                start=(k == 0), stop=(k == K - 1),
                                skip_group_check=True)
                # tails: sum 4 corner lanes + bias -> fp32 out (per half)
                for h in range(2):
                    pv = ps[h][:].rearrange("o (j n) -> o j n", j=4)
                    with tc.high_priority():
                        t = opool.tile([P, NH], f32, tag="t")
                        nc.vector.tensor_scalar(
                            out=t[:], in0=pv[:, 0, :],
                            scalar1=bias[:, 0:1], scalar2=None, op0=ALU.add)
                        t2 = opool.tile([P, NH], f32, tag="t2")
                        nc.vector.tensor_tensor(
                            out=t2[:], in0=t[:], in1=pv[:, 1, :], op=ALU.add)
                        t3 = opool.tile([P, NH], f32, tag="t3")
                        nc.vector.tensor_tensor(
                            out=t3[:], in0=t2[:], in1=pv[:, 2, :], op=ALU.add)
                        # final op reorders i' = 8p+q  ->  (q, p) blocks
                        o = opool.tile([P, NH], f32, tag="o")
                        nc.vector.tensor_tensor(
                            out=o[:].rearrange("o (q p) -> o p q", q=8, p=64),
                            in0=t3[:].rearrange("o (p q) -> o p q", p=64, q=8),
                            in1=pv[:, 3, :].rearrange("o (p q) -> o p q",
                                                      p=64, q=8),
                            op=ALU.add)
                    # store: o[(q,p)] -> out[qt*1024 + q*128 + h*64 + p]
                    eng = nc.sync if h == 0 else nc.scalar
                    eng.dma_start(
                        out=out_d.ap().rearrange(
                            "o (qt q hh p) -> o qt q hh p",
                            qt=4, q=8, hh=2, p=64)[:, qt, :, h, :],
                        in_=o[:].rearrange("o (q p) -> o q p", q=8, p=64))

    nc.compile()
    return nc


_NC_CACHE = None


def _host_inputs(x, offset, weight, bias):
    """Per-core input maps (core b <- batch b) + replicated constants."""
    wq = np.ascontiguousarray(weight, np.float32)  # [COUT, CIN, KH, KW]
    wmat = wq.reshape(COUT, CIN, K).transpose(1, 2, 0).reshape(CIN, K * COUT)
    wmat = np.ascontiguousarray(wmat).astype(ml_dtypes.bfloat16)
    bias_h = np.ascontiguousarray(bias, np.float32).reshape(P, 1)
    # hob[Pp, k*Q+q] = ho(p) - 1 + ky,  wob = wo(p) - 1 + kx,  p = q*128 + Pp
    p_of = (np.arange(Q)[:, None] * P + np.arange(P)[None, :])  # [Q, P]
    ho = (p_of // WO).astype(np.float32)
    wo = (p_of % WO).astype(np.float32)
    hobwob = np.empty((P, 2 * K * Q), np.float32)
    for k in range(K):
        hobwob[:, k * Q: (k + 1) * Q] = (ho + (k // 3 - 1)).T
        hobwob[:, K * Q + k * Q: K * Q + (k + 1) * Q] = (wo + (k % 3 - 1)).T
    in_maps = []
    for b in range(B):
        img = np.ascontiguousarray(x[b], np.float32).transpose(1, 2, 0)
        XPf = np.zeros((HP, WP, P), np.float32)
        XPf[PADR:PADR + H, PADR:PADR + W] = img
        ext = np.vstack([XPf.reshape(NE, P), np.zeros((WP + 1, P), np.float32)])
        xpm = np.concatenate(
            [ext[0:NE], ext[WP:NE + WP], ext[1:NE + 1], ext[WP + 1:NE + WP + 1]],
            axis=1).astype(ml_dtypes.bfloat16)  # [NE, 512]
        offb = np.ascontiguousarray(offset[b], np.float32).reshape(2 * K, NPOS)
        offt = np.empty((P, 2 * K * Q), np.float32)
        for k in range(K):
            offt[:, k * Q: (k + 1) * Q] = offb[2 * k].reshape(Q, P).T
            offt[:, K * Q + k * Q: K * Q + (k + 1) * Q] = \
                offb[2 * k + 1].reshape(Q, P).T
        in_maps.append({
            "xpm": xpm,
            "offt": offt,
            "hobwob": hobwob,
            "wmat": wmat,
            "bias": bias_h,
        })
    return in_maps


def kernel(x, offset, weight, bias):
    global _NC_CACHE
    from concourse.bass_utils import run_bass_kernel_spmd

    if _NC_CACHE is None:
        _NC_CACHE = _build_kernel()
    nc = _NC_CACHE
    in_maps = _host_inputs(x, offset, weight, bias)
    res = run_bass_kernel_spmd(nc, in_maps, list(range(B)))
    out = np.stack([res.results[b]["out"].reshape(COUT, HO, WO) for b in range(B)])
    return out.astype(np.float32)


if __name__ == "__main__":
    import sys
    d = np.load("/tmp/inputs.npz")
    if len(sys.argv) > 1 and sys.argv[1] == "sim":
        from concourse.bass_interp import CoreSim
        nc = _build_kernel()
        in_maps = _host_inputs(d["x"], d["offset"], d["weight"], d["bias"])
        sim = CoreSim(nc)
        for kk, vv in in_maps[0].items():
            sim.tensor(kk)[:] = vv
        sim.simulate()
        out = np.asarray(sim.tensor("out")).reshape(1, COUT, HO, WO)
        exp = np.load("/tmp/expected.npy")[0:1]
    else:
        out = kernel(d["x"], d["offset"], d["weight"], d["bias"])
        exp = np.load("/tmp/expected.npy")
    err = np.abs(out - exp)
    print("rel l2:", np.linalg.norm(out - exp) / np.linalg.norm(exp))
    print("absmax rel:", err.max() / np.abs(exp).max())
